# revision 13
# baseline (speedup 1.0000x reference)
"""ChamferLoss Trainium2 kernel.

reference:
    dist[b,n,m] = ||x[b,n]||^2 + ||y[b,m]||^2 - 2 x[b,n].y[b,m]
    out = sum_b mean_n min_m dist[b,n,m]

Strategy (8 cores, data-parallel over batch; 2 batches per core):
  On each core, per batch:
    D'[n,m] = x.y - 0.5||y||^2 - 0.5||x||^2 = -dist[n,m]/2
  computed by the PE as a K=13 fp16 matmul (hi/lo split for precision):
    lhsT rows: [xh0..2, xh0..2, xl0..2, 1, 1, ch, cl]  (ch+cl = -0.5||x||^2)
    rhs  rows: [yh0..2, yl0..2, yh0..2, zh, zl, 1, 1]  (zh+zl = -0.5||y||^2)
  Operand rows [13, 4096] are built on-chip: an interleaved fp16 stage tile
  feeds 32 per-block PE transposes ([128,13] -> [13,128]) into PSUM, then
  engine copies move them to SBUF rows (no single-partition flatten DMAs).
  max_m D' runs as a chained TensorTensorScan(max,max) on DVE: each scan
  consumes one PSUM tile (1024 cols) plus the ACT-cast fp16 copy of its
  partner tile, i.e. 2 columns/cycle on DVE with the running max carried
  in the fp32 scan state; the chain's last element is the tile's max.
  Host combines: out = sum(-2 * partials) / N.  v-order permutations of
  n and m are free (min/sum are permutation invariant).
"""

import sys

for _p in ("/opt/trn_rl_repo",):
    if _p not in sys.path:
        sys.path.insert(0, _p)

from contextlib import ExitStack

import numpy as np

import concourse.bass as bass
import concourse.mybir as mybir
import concourse.tile as tile
import concourse.bass_utils as _bu
from concourse.bass_utils import run_bass_kernel_spmd
from concourse.masks import make_identity
from concourse.vector_clock import ScopedClock

# Skip the BIR simulator during walrus compile (single-CPU container: birsim
# dominates compile time and is only a validation pass).
if not getattr(_bu, "_birsim_patched", False):
    _orig_run_command = _bu.run_command

    def _fast_run_command(argv, **kwargs):
        argv = [
            a.replace("--enable-birsim=true", "--enable-birsim=false")
            if isinstance(a, str)
            else a
            for a in argv
        ]
        return _orig_run_command(argv, **kwargs)

    _bu.run_command = _fast_run_command
    _bu._birsim_patched = True

# The walrus in this container rejects instructions carrying more than one
# sync wait ("Too many sync wait commands" in CoreV3GenImpl setupSyncWait).
# TileContext's final drain waits on every engine/queue semaphore at once.
# Split it: one drain instruction per wait.


def _split_drain_and_barrier(self, tick_clock, wait_clock):
    nc = self.nc
    drain_inst = nc.sync.drain()
    wait_clock.add_sem_waits(
        drain_inst.ins, ScopedClock({None: tick_clock.global_clock})
    )
    si = drain_inst.ins.sync_info
    waits = list(si.on_wait or []) if si is not None else []
    if len(waits) > 1:
        si.on_wait = waits[:1]
        for w in waits[1:]:
            d2 = nc.sync.drain()
            d2.ins.sync_info = mybir.SyncInfo(on_wait=[w], on_update=[])
    nc.all_engine_barrier()
    assert self.sems is not None
    popped = nc._tile_sem_poison_stack.pop()
    assert popped is self._sem_poison
    nc.clear_and_free_semaphores(list(self.sems.allocated().values()))
    nc.all_engine_barrier()


tile.TileContext._drain_and_barrier = _split_drain_and_barrier


def _fix_multiwait_json(nc) -> bytes:
    """Walrus here accepts at most one sync wait/update per instruction.

    Rewrite the serialized BIR: hoist excess waits onto single-wait Drain
    carrier instructions inserted just before the offending instruction (same
    engine, so program order preserves the wait semantics). Excess updates
    move onto a Drain inserted just after (engines/queues complete in order).
    """
    import orjson

    m = orjson.loads(nc.to_json_bytes())
    ctr = 0
    for f in m["functions"]:
        for blk in f["blocks"]:
            out = []
            for inst in blk["instructions"]:
                si = inst.get("sync_info")
                post = None
                if si:
                    waits = si.get("on_wait") or []
                    if len(waits) > 1:
                        for w in waits[:-1]:
                            ctr += 1
                            out.append(
                                {
                                    "name": f"I-wfix-{ctr}",
                                    "opcode": "Drain",
                                    "engine": inst["engine"],
                                    "ins": [],
                                    "outs": [],
                                    "is_reset_sema": False,
                                    "debug": inst.get("debug"),
                                    "sync_info": {
                                        "on_wait": [w],
                                        "on_update": [],
                                    },
                                }
                            )
                        si["on_wait"] = waits[-1:]
                    ups = si.get("on_update") or []
                    if len(ups) > 1:
                        ctr += 1
                        post = {
                            "name": f"I-ufix-{ctr}",
                            "opcode": "Drain",
                            "engine": inst["engine"],
                            "ins": [],
                            "outs": [],
                            "is_reset_sema": False,
                            "debug": inst.get("debug"),
                            "sync_info": {"on_wait": [], "on_update": ups[1:]},
                        }
                        si["on_update"] = ups[:1]
                out.append(inst)
                if post is not None:
                    out.append(post)
            blk["instructions"] = out
    return orjson.dumps(m)


B, N, D = 16, 4096, 3
NCORES = 8
BPC = B // NCORES  # batches per core
P = 128
W = 32  # point-blocks: point index = 32*p + w
K = 13  # operand rows
NT = 32  # n-tiles (one per x point-block)
PAIR = 2048  # m-columns per reduce step (2 psum tiles)
NPAIR = N // PAIR  # 2 pairs per n-tile
TPB = 8  # transposed blocks per tps psum tile (8*128 = 1024 cols)

f32 = mybir.dt.float32
f16 = mybir.dt.float16
AX = mybir.AxisListType.X
OP_MAX = mybir.AluOpType.max
OP_ADD = mybir.AluOpType.add
OP_SUB = mybir.AluOpType.subtract

NEG_INF = -3.0e38


def _emit(nc: bass.Bass, cfg: dict):
    """Emit the whole per-core program."""
    pattern = cfg["pattern"]  # per 2048-col pair: 'dve' (scan) or 'skip'
    cpat = cfg.get("cpat", "aaaaaaaa")  # tps->SBUF copy engines (v/a/g)

    xc = nc.declare_dram_parameter("xc", [BPC, N, D], f32, isOutput=False)
    yc = nc.declare_dram_parameter("yc", [BPC, N, D], f32, isOutput=False)
    out = nc.declare_dram_parameter("partial", [P, BPC], f32, isOutput=True)

    import contextlib

    rep = cfg.get("repeat", 1)

    with tile.TileContext(nc) as tc, ExitStack() as ctx:
        singles = ctx.enter_context(tc.tile_pool(name="singles", bufs=1))
        natp = ctx.enter_context(tc.tile_pool(name="natp", bufs=2))
        stgp = ctx.enter_context(tc.tile_pool(name="stgp", bufs=2))
        opsp = ctx.enter_context(tc.tile_pool(name="opsp", bufs=2))
        red = ctx.enter_context(tc.tile_pool(name="red", bufs=2))
        cstp = ctx.enter_context(tc.tile_pool(name="cstp", bufs=2))
        scrp = ctx.enter_context(tc.tile_pool(name="scrp", bufs=2))
        # One PSUM pool: 2 rotating 4-bank buffers (8KB/partition each) shared
        # by the matmul groups ([P,2048] f32) and, via the same tag, the
        # transpose staging ([P,4096] f16).
        pm = ctx.enter_context(
            tc.tile_pool(name="pm", bufs=cfg["pm_bufs"], space="PSUM")
        )

        ident = singles.tile([P, P], f16)
        make_identity(nc, ident)
        out_sb = singles.tile([P, BPC], f32)

        rep_cm = tc.For_i(0, rep, 1) if rep > 1 else contextlib.nullcontext()
        ctx.enter_context(rep_cm)

        copy_engines = {"v": nc.vector, "a": nc.scalar, "g": nc.gpsimd}
        cpat_seq = [c for c in cpat if c in copy_engines]

        for b in range(BPC):
            # ---- natural-layout loads: partition p holds points 32p..32p+31
            x_nat = natp.tile([P, 96], f32, tag="x_nat")
            y_nat = natp.tile([P, 96], f32, tag="y_nat")
            nc.sync.dma_start(
                out=x_nat, in_=xc[b].rearrange("(p w) d -> p (w d)", p=P)
            )
            nc.sync.dma_start(
                out=y_nat, in_=yc[b].rearrange("(p w) d -> p (w d)", p=P)
            )

            # ---- fp16 coords (hi/lo) + hi/lo split of -0.5*||.||^2 (DVE)
            def prep_side(nat_t, tagpfx):
                eng = nc.vector
                hi16 = natp.tile([P, 96], f16, tag=tagpfx + "hi16")
                eng.tensor_copy(out=hi16, in_=nat_t)
                lo16 = natp.tile([P, 96], f16, tag=tagpfx + "lo16")
                eng.tensor_sub(out=lo16, in0=nat_t, in1=hi16)
                # squared norm from the fp16 sum (hi+lo), fp32 arithmetic
                acc = natp.tile([P, 96], f32, tag=tagpfx + "acc")
                eng.tensor_add(out=acc, in0=hi16, in1=lo16)
                sq = natp.tile([P, 96], f32, tag=tagpfx + "sq")
                eng.tensor_mul(out=sq, in0=acc, in1=acc)
                sq3 = sq.rearrange("p (w d) -> p w d", d=D)
                t1 = natp.tile([P, 32], f32, tag=tagpfx + "t1")
                eng.tensor_add(out=t1, in0=sq3[:, :, 0], in1=sq3[:, :, 1])
                nrm = natp.tile([P, 32], f32, tag=tagpfx + "nrm")
                eng.tensor_add(out=nrm, in0=t1, in1=sq3[:, :, 2])
                eng.tensor_scalar_mul(out=nrm, in0=nrm, scalar1=-0.5)
                hi = natp.tile([P, 32], f16, tag=tagpfx + "hi")
                eng.tensor_copy(out=hi, in_=nrm)
                lo = natp.tile([P, 32], f16, tag=tagpfx + "lo")
                eng.scalar_tensor_tensor(
                    out=lo, in0=nrm, scalar=0.0, in1=hi, op0=OP_ADD, op1=OP_SUB
                )
                return hi16, lo16, hi, lo

            xh16, xl16, ch_nat, cl_nat = prep_side(x_nat, "x")
            yh16, yl16, zh_nat, zl_nat = prep_side(y_nat, "y")

            # ---- interleaved stage [P, W*K]: free index = w*13 + kk (Pool)
            def build_stage(hi16, lo16, nh, nl, roles, tag):
                st = stgp.tile([P, W * K], f16, tag=tag)
                nc.gpsimd.memset(st, 1.0)  # ones rows stay 1.0
                st3 = st.rearrange("p (w k) -> p w k", k=K)
                hi3 = hi16.rearrange("p (w d) -> p w d", d=D)
                lo3 = lo16.rearrange("p (w d) -> p w d", d=D)
                srcs = {"h": hi3, "l": lo3, "nh": nh, "nl": nl}
                for kk0, kk1, which in roles:
                    src = srcs[which]
                    if which in ("h", "l"):
                        nc.gpsimd.tensor_copy(out=st3[:, :, kk0:kk1], in_=src)
                    else:
                        nc.gpsimd.tensor_copy(out=st3[:, :, kk0], in_=src)
                return st

            stx = build_stage(
                xh16, xl16, ch_nat, cl_nat,
                [(0, 3, "h"), (3, 6, "h"), (6, 9, "l"), (11, 12, "nh"), (12, 13, "nl")],
                "stx",
            )
            sty = build_stage(
                yh16, yl16, zh_nat, zl_nat,
                [(0, 3, "h"), (3, 6, "l"), (6, 9, "h"), (9, 10, "nh"), (10, 11, "nl")],
                "sty",
            )

            # ---- per-block PE transposes -> PSUM, engine copies -> SBUF rows
            # ops layout (per side): [kk, w*128 + p]  == [13, 4096] rows
            opsx = opsp.tile([P, N], f16, tag="opsx")
            opsy = opsp.tile([P, N], f16, tag="opsy")
            ci = 0
            for side, st, ops in ((0, stx, opsx), (1, sty, opsy)):
                st3 = st.rearrange("p (w k) -> p w k", k=K)
                tps = pm.tile([P, N], f16, tag="ps", name=f"tps_{b}_{side}")
                for w in range(W):
                    nc.tensor.transpose(
                        out=tps[0:K, w * P : (w + 1) * P],
                        in_=st3[:, w, :],
                        identity=ident,
                    )
                eng = copy_engines[cpat_seq[ci % len(cpat_seq)]]
                ci += 1
                if eng is nc.scalar:
                    eng.copy(out=ops[0:K, :], in_=tps[0:K, :])
                else:
                    eng.tensor_copy(out=ops[0:K, :], in_=tps[0:K, :])

            # ---- main loop: matmul D' tiles + chained scan max-reduce
            res = red.tile([P, NT], f16, tag="res")
            pc = 0
            for nt in range(NT):
                lhsT = opsx[0:K, nt * P : (nt + 1) * P]
                pss = []
                for half in range(2):
                    ps = pm.tile(
                        [P, PAIR], f32, tag="ps", name=f"ps_{b}_{nt}_{half}"
                    )
                    for h in range(4):
                        m0 = half * PAIR + h * 512
                        nc.tensor.matmul(
                            ps[:, h * 512 : (h + 1) * 512],
                            lhsT,
                            opsy[0:K, m0 : m0 + 512],
                            start=True,
                            stop=True,
                        )
                    pss.append(ps)
                mode = pattern[pc % len(pattern)]
                pc += 1
                if mode == "skip":
                    nc.gpsimd.memset(res[:, nt : nt + 1], 0.0)
                    continue
                # ACT casts the first-filled half; one DVE scan consumes the
                # second PSUM half + the cast, carrying the running max in
                # its fp32 state. Its last element is this tile's max.
                ca = cstp.tile([P, PAIR], f16, tag="ca")
                nc.scalar.copy(out=ca, in_=pss[0])
                scr = scrp.tile([P, PAIR], f16, tag="scr")
                nc.vector.tensor_tensor_scan(
                    out=scr,
                    data0=pss[1],
                    data1=ca,
                    initial=NEG_INF,
                    op0=OP_MAX,
                    op1=OP_MAX,
                )
                nc.gpsimd.tensor_copy(
                    out=res[:, nt : nt + 1], in_=scr[:, PAIR - 1 : PAIR]
                )
            bsum = red.tile([P, 1], f32, tag="bsum")
            nc.vector.reduce_sum(out=bsum, in_=res, axis=AX)
            nc.vector.tensor_copy(out=out_sb[:, b : b + 1], in_=bsum)

        nc.sync.dma_start(out=out[:, :], in_=out_sb)


_CACHE = {}

DEFAULT_CFG = {"pm_bufs": 2, "pattern": ("dve",), "cpat": "aa"}


def _get_program(cfg_key=None):
    cfg = dict(DEFAULT_CFG)
    if cfg_key:
        cfg.update(cfg_key)
    key = tuple(sorted((k, str(v)) for k, v in cfg.items()))
    if key not in _CACHE:
        nc = bass.Bass(trn_type="TRN2", debug=False)
        _emit(nc, cfg)
        fixed = _fix_multiwait_json(nc)
        nc.to_json_bytes = lambda: fixed
        _CACHE[key] = nc
    return _CACHE[key]


def kernel(x: np.ndarray, y: np.ndarray, _cfg=None, _trace=False):
    assert x.shape == (B, N, D) and y.shape == (B, N, D)
    nc = _get_program(_cfg)
    in_maps = []
    for c in range(NCORES):
        sl = slice(c * BPC, (c + 1) * BPC)
        in_maps.append(
            {
                "xc": np.ascontiguousarray(x[sl], dtype=np.float32),
                "yc": np.ascontiguousarray(y[sl], dtype=np.float32),
            }
        )
    kw = {}
    if _trace:
        kw = {"trace": True, "trace_cores": list(range(NCORES)), "stitch_traces": False}
    res = run_bass_kernel_spmd(nc, in_maps, core_ids=list(range(NCORES)), **kw)
    total = 0.0
    for r in res.results:
        total += np.asarray(r["partial"], dtype=np.float64).sum()
    val = np.float32(-2.0 * total / N)
    if _trace:
        return np.array(val, dtype=np.float32), res
    return np.array(val, dtype=np.float32)
